# revision 39
# baseline (speedup 1.0000x reference)
"""Trainium2 Bass kernel for a dense transformer block (nn_Block_3453153706485).

B=4, S=1024, D=1024, H=16 heads (hd=64), FF=4096, fp32 I/O.
Sharding: 8 cores; core c owns (batch b=c//2, token half c%2) -> 512 query
tokens. LN1 + k/v are computed for the full batch on both cores of a pair
(duplicated) so no cross-core communication is needed.

Per-core pipeline (static SPMD; per-core differences live in the input data):
  A:  LN1(x_full) -> h (bf16) -> PE-transpose -> hT fp8e4 [D, 1024]
  B:  k/v/q projections via fp8e4 DoubleRow matmuls against x32-scaled
      weights; RoPE (cos/sin tables pre-scaled by 1/32) -> kT/qT bf16;
      v1 fp8e4 [tok, head, 66] with col 64 = 1 (softmax denominator row).
  C:  per head pair: scoresT = kT_h.T @ qT_h (bf16, row-tiled 2 heads);
      exp(s/8 + mask_bias) -> pT fp8e4 (mask folded into the per-partition
      activation bias: masked keys get -30 -> exp ~ 0);
      pv via DoubleRow: out2 = v1_h.T @ pT; row 64 = Z; 1/Z via
      reciprocal_approx_fast on the psum row, partition-broadcast, and the
      normalize is fused into the psum->sbuf eviction -> attnT fp8e4.
  D:  wo (DoubleRow) + residual via scalar_tensor_tensor (x 1/32 folded in);
      LN2 -> h2 bf16 -> h2T fp8e4
  E:  MLP via DoubleRow fp8; gelu applies the 1/32 w1 descale; final
      residual add applies the 1/32 w2 descale via scalar_tensor_tensor.
"""

from contextlib import ExitStack

import ml_dtypes
import numpy as np

import concourse.bass as bass
import concourse.tile as tile
from concourse import bacc, mybir
from concourse.masks import make_identity

F32 = mybir.dt.float32
F32R = mybir.dt.float32r
BF16 = mybir.dt.bfloat16
F8 = mybir.dt.float8e4
AF = mybir.ActivationFunctionType
OP = mybir.AluOpType
DR = mybir.MatmulPerfMode.DoubleRow

P = 128
D = 1024
H = 16
HD = 64
FF = 4096
TB = 1024          # tokens per batch
TL = 512           # local (query) tokens per core
KC = D // P        # 8
NCORES = 8
EPS = 1e-5
WS = 32.0          # fp8 weight upscale
RWS = float(1.0 / WS)
MBIAS = -30.0      # exp bias for masked keys


def _swap_pairs(ap4):
    """View with the two elements of each innermost [step,2] pair swapped."""
    st = ap4.ap[-1][0]
    return bass.AP(
        tensor=ap4.tensor,
        offset=ap4.offset + st,
        ap=list(ap4.ap[:-1]) + [[-st, 2]],
    )


def build_program(apply_ln1: bool, apply_ln2: bool, sim_compat: bool = False,
                  dbg: bool = False):
    nc = bacc.Bacc("TRN2", target_bir_lowering=False, debug=False)

    xf = nc.dram_tensor("xf", [TB, D], F32, kind="ExternalInput").ap()
    xq = nc.dram_tensor("xq", [TL, D], F32, kind="ExternalInput").ap()
    mb_d = nc.dram_tensor("mb", [P, TB // P], F32, kind="ExternalInput").ap()
    cosf_d = nc.dram_tensor("cosf", [P, TB // P, HD], F32, kind="ExternalInput").ap()
    sinf_d = nc.dram_tensor("sinf", [P, TB // P, HD], F32, kind="ExternalInput").ap()
    wq_d = nc.dram_tensor("wq", [P, KC, D], F8, kind="ExternalInput").ap()
    wk_d = nc.dram_tensor("wk", [P, KC, D], F8, kind="ExternalInput").ap()
    wv_d = nc.dram_tensor("wv", [P, KC, D], F8, kind="ExternalInput").ap()
    wo_d = nc.dram_tensor("wo", [P, KC, D], F8, kind="ExternalInput").ap()
    w1_d = nc.dram_tensor("w1", [P, KC, FF], BF16, kind="ExternalInput").ap()
    w2_d = nc.dram_tensor("w2", [P, FF // P, D], F8, kind="ExternalInput").ap()
    ln_d = {nm: nc.dram_tensor(nm, [1, D], F32, kind="ExternalInput").ap()
            for nm in ("ln1w", "ln1b", "ln2w", "ln2b")}
    out_d = nc.dram_tensor("out", [TL, D], F32, kind="ExternalOutput").ap()
    dbg_d = {}
    if dbg:
        for nm, shape, dt in [
                ("d_hT", [P, KC, TB], F8), ("d_kT", [P, KC, TB], BF16),
                ("d_qT", [P, KC, TL], BF16), ("d_v1", [P, TB // P, H, 66], F8),
                ("d_pt0", [P, KC, 2, 512], F8), ("d_at", [P, KC, TL], F8),
                ("d_xr", [P, TL // P, D], F32), ("d_h2T", [P, KC, TL], BF16),
                ("d_g1", [P, FF // P, TL], F8), ("d_z", [1, D], F32),
                ("d_rz", [1, D], F32)]:
            dbg_d[nm] = nc.dram_tensor(nm, shape, dt,
                                       kind="ExternalOutput").ap()

    gelu_f = AF.Identity if sim_compat else AF.Gelu

    with tile.TileContext(nc) as tc:
        es0 = ExitStack()

        cons = es0.enter_context(tc.tile_pool(name="cons", bufs=1))
        work = es0.enter_context(tc.tile_pool(name="work", bufs=2))

        # ---------------- constants ----------------
        identb = cons.tile([P, P], BF16, name="identb")
        make_identity(nc, identb)
        eps_t = cons.tile([P, 1], F32, name="eps_t")
        nc.vector.memset(eps_t[:], EPS)
        # constants ride the gpsimd queue so xt loads own sync+scalar queues
        mbias = cons.tile([P, TB // P], F32, name="mb_sb")
        nc.gpsimd.dma_start(mbias[:], mb_d)
        cosf = cons.tile([P, TB // P, HD], F32, name="cosf_sb")
        nc.gpsimd.dma_start(cosf[:], cosf_d)
        sinf = cons.tile([P, TB // P, HD], F32, name="sinf_sb")
        nc.gpsimd.dma_start(sinf[:], sinf_d)

        ln_bc = {}
        for nm, need in (("ln1w", apply_ln1), ("ln1b", apply_ln1),
                         ("ln2w", apply_ln2), ("ln2b", apply_ln2)):
            if need:
                t = cons.tile([P, D], F32, name=f"{nm}_bc")
                src = bass.AP(tensor=ln_d[nm].tensor, offset=ln_d[nm].offset,
                              ap=[[0, P], [1, D]])
                nc.sync.dma_start(t[:], src)
                ln_bc[nm] = t

        def layernorm(src_ap, dst_ap, wkey, bkey, applied):
            stats = work.tile([P, 2, 6], F32, tag="stats", name="stats")
            nc.vector.bn_stats(stats[:, 0, :], src_ap[:, 0:512])
            nc.vector.bn_stats(stats[:, 1, :], src_ap[:, 512:1024])
            mv = work.tile([P, 2], F32, tag="mv", name="mv")
            nc.vector.bn_aggr(mv[:], stats[:])
            std = work.tile([P, 1], F32, tag="std", name="std")
            nc.scalar.activation(std[:], mv[:, 1:2], AF.Sqrt, bias=eps_t[:])
            rstd = work.tile([P, 1], F32, tag="rstd", name="rstd")
            nc.vector.reciprocal(rstd[:], std[:])
            nc.vector.tensor_scalar(dst_ap, src_ap, mv[:, 0:1], rstd[:],
                                    OP.subtract, OP.mult)
            if applied:
                nc.gpsimd.tensor_mul(dst_ap, dst_ap, ln_bc[wkey][:])
                nc.gpsimd.tensor_add(dst_ap, dst_ap, ln_bc[bkey][:])

        # PSUM pool for stages A+B:
        #   tp: bf16 [P,1024] transpose staging (1 bank x2)
        #   pj: f32 [P,1024] projection accumulate (2 banks x2)
        psAB = tc.alloc_tile_pool(name="psAB", bufs=1, space="PSUM")

        def transpose8b(h_ap, dst_ap, evict):
            """h_ap [128, 1024] bf16 -> dst_ap [128, 8, 128] via PE (bf16)."""
            pst = psAB.tile([P, D], BF16, tag="tp", bufs=2, name="pst")
            for c in range(KC):
                nc.tensor.transpose(pst[:, c * P:(c + 1) * P],
                                    h_ap[:, c * P:(c + 1) * P], identb[:])
            evict(dst_ap, pst.rearrange("p (k t) -> p k t", k=KC))

        def proj_dr(ps_half, lhs3, wtile):
            """DoubleRow projection: lhs3 [P, KC, 128] fp8, wtile [P, KC, 512]
            fp8 -> ps_half [128, 512] f32."""
            for k2 in range(KC // 2):
                nc.tensor.matmul(
                    ps_half, lhsT=lhs3[:, 2 * k2:2 * k2 + 2, :],
                    rhs=wtile[:, 2 * k2:2 * k2 + 2, :],
                    start=(k2 == 0), stop=(k2 == KC // 2 - 1),
                    perf_mode=DR)

        # ========== stage A: LN1 of full batch -> hT (fp8) ==========
        pool_hT = tc.alloc_tile_pool(name="p_hT", bufs=1, side="right")
        hT = pool_hT.tile([P, KC, TB], F8, name="hT")
        for ti in range(TB // P):
            xt = work.tile([P, D], F32, tag="xt", name=f"xt{ti}")
            dma_eng = nc.sync if ti % 2 == 0 else nc.scalar
            dma_eng.dma_start(xt[:], xf[ti * P:(ti + 1) * P, :])
            h = work.tile([P, D], BF16, tag="h", name=f"h{ti}")
            layernorm(xt[:], h[:], "ln1w", "ln1b", apply_ln1)
            transpose8b(h[:], hT[:, :, ti * P:(ti + 1) * P], nc.scalar.copy)

        # ========== stage B: k, v (full batch), q (local) ==========
        pool_wB = tc.alloc_tile_pool(name="p_wB", bufs=2, side="right")
        pool_kT = tc.alloc_tile_pool(name="p_kT", bufs=1)
        kT = pool_kT.tile([P, KC, TB], BF16, name="kT")

        def rope1024(ps, cos_sb, sin_sb, ti, dst_ap):
            """RoPE a [128, 1024] psum projection tile (16 heads) -> bf16."""
            ps_h = ps.rearrange("p (h i) -> p h i", h=H)
            cos_b = cos_sb[:, ti, None, :].to_broadcast((P, H, HD))
            p1 = work.tile([P, D], BF16, tag="p1", bufs=2, name="p1")
            nc.vector.tensor_tensor(p1.rearrange("p (h i) -> p h i", h=H),
                                    ps_h, cos_b, OP.mult)
            ps_sw = _swap_pairs(ps.rearrange("p (h i two) -> p h i two",
                                             h=H, two=2))
            sin_b = (sin_sb[:, ti, None, :].to_broadcast((P, H, HD))
                     .rearrange("p h (i two) -> p h i two", two=2))
            p2 = work.tile([P, D], BF16, tag="p2", bufs=2, name="p2")
            nc.vector.tensor_tensor(
                p2.rearrange("p (h i two) -> p h i two", h=H, two=2),
                ps_sw, sin_b, OP.mult)
            nc.vector.tensor_add(dst_ap, p1[:], p2[:])

        def flushb(pend):
            kr, dstT, ti = pend
            pst = psAB.tile([P, D], BF16, tag="tp", bufs=2, name="fls")
            for c in range(KC):
                nc.tensor.transpose(pst[:, c * P:(c + 1) * P],
                                    kr[:, c * P:(c + 1) * P], identb[:])
            nc.vector.tensor_copy(dstT[:, :, ti * P:(ti + 1) * P],
                                  pst.rearrange("p (k t) -> p k t", k=KC))

        # q path first (reuses hT -- no duplicate LN/transpose) so qT is
        # complete before scores start.  Tokens are rolled per core so the
        # local 512 query tokens are tiles 0..3 of the (rolled) batch;
        # attention is order-invariant over keys.
        pool_qT = tc.alloc_tile_pool(name="p_qT", bufs=1)
        qT = pool_qT.tile([P, KC, TL], BF16, name="qT")
        wqh = []
        for nh in range(2):
            t = pool_wB.tile([P, KC, 512], F8, tag="wB", name=f"wq{nh}")
            nc.gpsimd.dma_start(t[:], wq_d[:, :, nh * 512:(nh + 1) * 512])
            wqh.append(t)
        pend = None
        for tiq in range(TL // P):
            ps = psAB.tile([P, D], F32, tag="pj", bufs=1, name=f"qps{tiq}")
            for nh in range(2):
                proj_dr(ps[:, nh * 512:(nh + 1) * 512],
                        hT[:, :, tiq * P:(tiq + 1) * P], wqh[nh])
            qr = work.tile([P, D], BF16, tag="kr", bufs=3, name=f"qr{tiq}")
            rope1024(ps, cosf, sinf, tiq, qr[:])
            if pend is not None:
                flushb(pend)
            pend = (qr, qT, tiq)
        flushb(pend)
        pend = None

        # pT for all 8 head pairs persists across the k/v loops: scores+exp
        # are emitted chunk-major, interleaved with the k projections, so the
        # 65us of exp runs under stage-B PE work instead of a separate phase
        pool_pT = tc.alloc_tile_pool(name="p_pT", bufs=1)
        ptA = pool_pT.tile([P, H // 2, KC, 2, 512], F8, name="ptA")

        def emit_scores(skc):
            """Scores + exp for sk-chunk skc, all 8 head pairs."""
            for j in range(H // 2):
                pss = psAB.tile([P, D], F32, tag="sc", bufs=2,
                                name=f"sc{j}_{skc}")
                for eo in range(2):
                    pb = 64 * eo
                    nc.tensor.matmul(
                        pss[:, eo * 512:(eo + 1) * 512],
                        lhsT=kT[pb:pb + 64, j, skc * P:(skc + 1) * P],
                        rhs=qT[pb:pb + 64, j, :],
                        start=True, stop=True)
                nc.scalar.activation(
                    ptA[:, j, skc, :, :].rearrange("p a b -> p (a b)"),
                    pss[:], AF.Exp, scale=0.125,
                    bias=mbias[:, skc:skc + 1])

        # k path, with scores/exp for chunk ti-2 interleaved
        wkh = []
        for nh in range(2):
            t = pool_wB.tile([P, KC, 512], F8, tag="wB", name=f"wk{nh}")
            nc.gpsimd.dma_start(t[:], wk_d[:, :, nh * 512:(nh + 1) * 512])
            wkh.append(t)
        for ti in range(TB // P):
            ps = psAB.tile([P, D], F32, tag="pj", bufs=1, name=f"kps{ti}")
            for nh in range(2):
                proj_dr(ps[:, nh * 512:(nh + 1) * 512],
                        hT[:, :, ti * P:(ti + 1) * P], wkh[nh])
            kr = work.tile([P, D], BF16, tag="kr", bufs=3, name=f"kr{ti}")
            rope1024(ps, cosf, sinf, ti, kr[:])
            if pend is not None:
                flushb(pend)
            pend = (kr, kT, ti)
            if ti >= 2:
                emit_scores(ti - 2)
        flushb(pend)
        pend = None

        # v path + remaining scores chunks
        pool_v1 = tc.alloc_tile_pool(name="p_v1", bufs=1)
        v1 = pool_v1.tile([P, TB // P, H, 66], F8, name="v1")
        nc.vector.memset(v1[:, :, :, 64:65], 1.0)
        nc.vector.memset(v1[:, :, :, 65:66], 0.0)
        wvh = []
        for nh in range(2):
            t = pool_wB.tile([P, KC, 512], F8, tag="wB", name=f"wv{nh}")
            nc.gpsimd.dma_start(t[:], wv_d[:, :, nh * 512:(nh + 1) * 512])
            wvh.append(t)
        for ti in range(TB // P):
            ps = psAB.tile([P, D], F32, tag="pj", bufs=1, name=f"vps{ti}")
            for nh in range(2):
                proj_dr(ps[:, nh * 512:(nh + 1) * 512],
                        hT[:, :, ti * P:(ti + 1) * P], wvh[nh])
            nc.vector.tensor_scalar_mul(
                v1[:, ti, :, 0:64],
                ps.rearrange("p (h d) -> p h d", h=H), float(RWS))
            if ti < 2:
                emit_scores(6 + ti)

        if dbg:
            nc.sync.dma_start(dbg_d["d_hT"], hT[:])
            nc.sync.dma_start(dbg_d["d_kT"], kT[:])
            nc.sync.dma_start(dbg_d["d_qT"], qT[:])
            nc.sync.dma_start(dbg_d["d_v1"], v1[:])

        pool_wB.release()
        pool_hT.release()
        psAB.release()

        # ========== stage C: pv + normalize per head pair ==========
        psC = tc.alloc_tile_pool(name="psC", bufs=1, space="PSUM")
        pool_wD = tc.alloc_tile_pool(name="p_wD", bufs=2, side="right")
        woh = []
        for nh in range(2):
            t = pool_wD.tile([P, KC, 512], F8, tag="wD", name=f"wo{nh}")
            nc.gpsimd.dma_start(t[:], wo_d[:, :, nh * 512:(nh + 1) * 512])
            woh.append(t)
        pool_at = tc.alloc_tile_pool(name="p_at", bufs=1, side="right")
        attnT = pool_at.tile([P, KC, TL], F8, name="attnT")

        def finish_pair(pj, pvps):
            # recip_approx_fast misreads PSUM on HW -- stage Z through SBUF
            zsb = work.tile([1, D], F32, tag="zsb", name=f"zsb{pj}")
            nc.vector.tensor_copy(zsb[0:1, :], pvps[64:65, :])
            rz = work.tile([1, D], F32, tag="rz", name=f"rz{pj}")
            nc.vector.reciprocal_approx_fast(rz[0:1, :], zsb[0:1, :])
            if dbg and pj == 0:
                nc.sync.dma_start(dbg_d["d_z"], zsb[0:1, :])
                nc.sync.dma_start(dbg_d["d_rz"], rz[0:1, :])
            # partition_broadcast only writes base-0 outputs: broadcast each
            # half across all 128 partitions and slice the needed rows
            rzbc = work.tile([P, 2, 512], F32, tag="rzbc", name=f"rzbc{pj}")
            nc.gpsimd.partition_broadcast(rzbc[:, 0, :], rz[0:1, 0:512])
            nc.gpsimd.partition_broadcast(rzbc[:, 1, :], rz[0:1, 512:1024])
            nc.vector.tensor_tensor(attnT[0:64, pj, :], pvps[0:64, 0:512],
                                    rzbc[0:64, 0, :], OP.mult)
            nc.vector.tensor_tensor(attnT[64:128, pj, :],
                                    pvps[0:64, 512:1024],
                                    rzbc[64:128, 1, :], OP.mult)

        prev = None
        for j in range(H // 2 + 1):
            last = j == H // 2
            if not last:
                pvps = psC.tile([P, D], F32, tag="pv", bufs=3,
                                name=f"pv{j}")
                for k2 in range(4):
                    for eo in range(2):
                        nc.tensor.matmul(
                            pvps[0:66, eo * 512:(eo + 1) * 512],
                            lhsT=v1[:, 2 * k2:2 * k2 + 2, 2 * j + eo, :],
                            rhs=ptA[:, j, 2 * k2:2 * k2 + 2, eo, :],
                            start=(k2 == 0), stop=(k2 == 3),
                            perf_mode=DR)
            if prev is not None:
                finish_pair(prev[0], prev[1])
            if not last:
                prev = (j, pvps)
        if dbg:
            nc.sync.dma_start(dbg_d["d_pt0"], ptA[:, 0, :, :, :])

        pool_v1.release()
        pool_pT.release()
        pool_qT.release()
        pool_kT.release()
        psC.release()

        # ========== stage D: wo + residual -> xres; LN2 -> h2T ==========
        psD = tc.alloc_tile_pool(name="psD", bufs=1, space="PSUM")
        pool_res = tc.alloc_tile_pool(name="p_res", bufs=1)
        xres = pool_res.tile([P, TL // P, D], F32, name="xres")
        h2T = pool_res.tile([P, KC, TL], BF16, name="h2T")
        pool_w1 = tc.alloc_tile_pool(name="p_w1", bufs=2)

        def transpose8d(h_ap, dst_ap):
            pst = psD.tile([P, D], BF16, tag="tp", bufs=2, name="pstD")
            for c in range(KC):
                nc.tensor.transpose(pst[:, c * P:(c + 1) * P],
                                    h_ap[:, c * P:(c + 1) * P], identb[:])
            nc.scalar.copy(dst_ap, pst.rearrange("p (k t) -> p k t", k=KC))

        pend_h2 = None
        for tc4 in range(TL // P):
            xtq = work.tile([P, D], F32, tag="xt", name=f"xtd{tc4}")
            nc.sync.dma_start(xtq[:], xq[tc4 * P:(tc4 + 1) * P, :])
            for nh in range(2):
                ps = psD.tile([P, 512], F32, tag="wo", bufs=3,
                              name=f"wops{tc4}_{nh}")
                proj_dr(ps[:], attnT[:, :, tc4 * P:(tc4 + 1) * P], woh[nh])
                nc.vector.scalar_tensor_tensor(
                    xres[:, tc4, nh * 512:(nh + 1) * 512], ps[:], RWS,
                    xtq[:, nh * 512:(nh + 1) * 512], OP.mult, OP.add)
            h2 = work.tile([P, D], BF16, tag="h", name=f"h2{tc4}")
            layernorm(xres[:, tc4, :], h2[:], "ln2w", "ln2b", apply_ln2)
            if pend_h2 is not None:
                ph2, ptc = pend_h2
                transpose8d(ph2[:], h2T[:, :, ptc * P:(ptc + 1) * P])
            pend_h2 = (h2, tc4)
        ph2, ptc = pend_h2
        transpose8d(ph2[:], h2T[:, :, ptc * P:(ptc + 1) * P])

        if dbg:
            nc.sync.dma_start(dbg_d["d_at"], attnT[:])
            nc.sync.dma_start(dbg_d["d_xr"], xres[:])
            nc.sync.dma_start(dbg_d["d_h2T"], h2T[:])

        pool_at.release()
        pool_wD.release()
        psD.release()

        # ========== stage E: MLP ==========
        psE = tc.alloc_tile_pool(name="psE", bufs=1, space="PSUM")
        pool_g1 = tc.alloc_tile_pool(name="p_g1", bufs=1, side="right")
        pool_w2 = tc.alloc_tile_pool(name="p_w2", bufs=2, side="right")
        g1 = pool_g1.tile([P, FF // P, TL], F8, name="g1")

        for fg in range(FF // 512):
            w1c = pool_w1.tile([P, KC, 512], BF16, tag="w1c", name=f"w1c{fg}")
            nc.gpsimd.dma_start(w1c[:], w1_d[:, :, fg * 512:(fg + 1) * 512])
            for jj in range(2):
                ps = psE.tile([P, D], F32, tag="m1", bufs=2,
                              name=f"m1ps{fg}_{jj}")
                for jh in range(2):
                    jx = 2 * jj + jh
                    for kc in range(KC):
                        nc.tensor.matmul(
                            ps[:, jh * 512:(jh + 1) * 512],
                            lhsT=w1c[:, kc, jx * P:(jx + 1) * P],
                            rhs=h2T[:, kc, :],
                            start=(kc == 0), stop=(kc == KC - 1))
                nc.scalar.activation(
                    g1[:, fg * 4 + 2 * jj:fg * 4 + 2 * jj + 2, :]
                    .rearrange("p a b -> p (a b)"),
                    ps[:], gelu_f)

        for nh in range(2):
            w2cs = []
            for kg in range(4):
                w2c = pool_w2.tile([P, 8, 512], F8, tag="w2c", bufs=5,
                                   name=f"w2c{nh}_{kg}")
                nc.gpsimd.dma_start(
                    w2c[:], w2_d[:, kg * 8:(kg + 1) * 8,
                                 nh * 512:(nh + 1) * 512])
                w2cs.append(w2c)
            # token-chunk-major so each output chain finishes (and DMAs out)
            # while later chunks still run on the PE
            for tc4 in range(TL // P):
                ps = psE.tile([P, 512], F32, tag="m2", bufs=2,
                              name=f"m2ps{nh}_{tc4}")
                for kg in range(4):
                    for k2 in range(4):
                        nc.tensor.matmul(
                            ps,
                            lhsT=g1[:, kg * 8 + 2 * k2:kg * 8 + 2 * k2 + 2,
                                    tc4 * P:(tc4 + 1) * P],
                            rhs=w2cs[kg][:, 2 * k2:2 * k2 + 2, :],
                            start=(kg == 0 and k2 == 0),
                            stop=(kg == 3 and k2 == 3),
                            perf_mode=DR)
                ot = work.tile([P, 512], F32, tag="osb", name=f"ot{nh}_{tc4}")
                nc.vector.scalar_tensor_tensor(
                    ot[:], ps[:], RWS,
                    xres[:, tc4, nh * 512:(nh + 1) * 512], OP.mult, OP.add)
                nc.sync.dma_start(out_d[tc4 * P:(tc4 + 1) * P,
                                        nh * 512:(nh + 1) * 512], ot[:])

        if dbg:
            nc.sync.dma_start(dbg_d["d_g1"], g1[:])

        pool_w2.release()
        pool_g1.release()
        pool_w1.release()
        pool_res.release()
        psE.release()
        es0.close()

    nc.compile()
    return nc


# ---------------------------------------------------------------------------
# Host side
# ---------------------------------------------------------------------------

_PROGRAM_CACHE = {}


def _get_program(apply_ln1, apply_ln2, sim_compat=False, dbg=False):
    key = (apply_ln1, apply_ln2, sim_compat, dbg)
    if key not in _PROGRAM_CACHE:
        _PROGRAM_CACHE[key] = build_program(*key)
    return _PROGRAM_CACHE[key]


def _prep_inputs(x, mask, freqs_cos, freqs_sin, wq, wk, wv, wo, w1, w2,
                 ln1_w, ln1_b, ln2_w, ln2_b):
    """Build the 8 per-core input dicts."""
    f32 = np.float32
    fp8 = ml_dtypes.float8_e4m3
    x = np.asarray(x, f32)
    mask = np.asarray(mask)
    cos = np.asarray(freqs_cos, f32)
    sin = np.asarray(freqs_sin, f32)

    ci = np.empty((TB, HD), f32)
    ci[:, 0::2] = cos
    ci[:, 1::2] = cos
    si = np.empty((TB, HD), f32)
    si[:, 0::2] = -sin
    si[:, 1::2] = sin
    ci *= RWS
    si *= RWS

    def tok_layout(t):
        """[TB, n] -> [P, TB//P, n] tile-major."""
        return np.ascontiguousarray(
            t.reshape(TB // P, P, -1).transpose(1, 0, 2))

    def wlayout(w, kc):
        w = np.asarray(w, f32)
        return np.ascontiguousarray(
            (w * WS).reshape(kc, P, w.shape[1]).transpose(1, 0, 2)).astype(fp8)

    def wlayout16(w, kc):
        w = np.asarray(w, f32)
        return np.ascontiguousarray(
            w.reshape(kc, P, w.shape[1]).transpose(1, 0, 2)).astype(
                ml_dtypes.bfloat16)

    shared = {
        "wq": wlayout(wq, KC), "wk": wlayout(wk, KC), "wv": wlayout(wv, KC),
        "wo": wlayout(wo, KC), "w1": wlayout16(w1, KC),
        "w2": wlayout(w2, FF // P),
        "ln1w": np.asarray(ln1_w, f32).reshape(1, D),
        "ln1b": np.asarray(ln1_b, f32).reshape(1, D),
        "ln2w": np.asarray(ln2_w, f32).reshape(1, D),
        "ln2b": np.asarray(ln2_b, f32).reshape(1, D),
    }

    in_maps = []
    for c in range(NCORES):
        b, half = divmod(c, 2)
        roll = half * TL
        mb = np.where(mask[b], MBIAS, 0.0).astype(f32)
        m = dict(shared)
        # roll tokens so the local 512 query tokens come first; key order is
        # irrelevant to attention as long as rope/mask/v follow the same roll
        m["xf"] = np.ascontiguousarray(np.roll(x[b], -roll, axis=0))
        m["xq"] = np.ascontiguousarray(x[b, half * TL:(half + 1) * TL])
        m["mb"] = np.ascontiguousarray(tok_layout(np.roll(mb, -roll))[:, :, 0])
        m["cosf"] = tok_layout(np.roll(ci, -roll, axis=0))
        m["sinf"] = tok_layout(np.roll(si, -roll, axis=0))
        in_maps.append(m)
    return in_maps


def kernel(x, mask, freqs_cos, freqs_sin, wq, wk, wv, wo, w1, w2,
           ln1_w, ln1_b, ln2_w, ln2_b, _trace=False, _sim=False, _dbg=False):
    from concourse.bass_utils import run_bass_kernel_spmd

    apply_ln1 = not (np.all(np.asarray(ln1_w) == 1.0)
                     and np.all(np.asarray(ln1_b) == 0.0))
    apply_ln2 = not (np.all(np.asarray(ln2_w) == 1.0)
                     and np.all(np.asarray(ln2_b) == 0.0))
    nc = _get_program(apply_ln1, apply_ln2,
                      sim_compat=(_sim is not False and _sim is not None),
                      dbg=_dbg)
    in_maps = _prep_inputs(x, mask, freqs_cos, freqs_sin, wq, wk, wv, wo,
                           w1, w2, ln1_w, ln1_b, ln2_w, ln2_b)

    if _sim is not False and _sim is not None:
        cidx = 0 if _sim is True else int(_sim)
        from concourse.bass_interp import CoreSim
        sim = CoreSim(nc, trace=False)
        for k, v in in_maps[cidx].items():
            sim.tensor(k)[:] = v
        sim.simulate(check_with_hw=False)
        b, half = divmod(cidx, 2)
        full = np.zeros((4, TB, D), np.float32)
        full[b, half * TL:(half + 1) * TL] = np.array(sim.tensor("out"))
        return full

    res = run_bass_kernel_spmd(nc, in_maps, core_ids=list(range(NCORES)),
                               trace=_trace)
    full = np.empty((4, TB, D), np.float32)
    for c in range(NCORES):
        b, half = divmod(c, 2)
        full[b, half * TL:(half + 1) * TL] = res.results[c]["out"]
    if _trace:
        return full, res
    return full
